# revision 7
# baseline (speedup 1.0000x reference)
"""Trainium2 Bass kernel for nn_CVQuantumLayer.

The reference "CV quantum circuit" evolves Gaussian means through
displacement / squeezing / beamsplitter gates. Every gate is affine in the
means vector (mx, mp), and the initial means are linear in x, so the whole
circuit collapses to an affine map per sample:

    out = concat(mx_circuit0(x), mp_circuit1(x)) = x @ W + b,   W [16, 32]

W and b are computed on host in float64 from the tiny gate parameters; the
heavy [1M, 16] @ [16, 32] + b map runs on 8 NeuronCores, data-parallel over
the batch.

Per-core layout: the 131072x16 fp32 shard is viewed as [16384 rows, 128]
(8 samples concatenated per row).  W is expanded to a block-diagonal
W_big [128, 256] (8 copies), so each [128, 128] tile of rows needs one
TensorE transpose (features onto partitions) + one 128-contraction matmul:

    tile [128 rows, 128 featcat] --PE transpose--> xT in PSUM
    xT --ACT copy--> SBUF
    matmul(lhsT=xT, rhs=W_big) -> out PSUM [128 rows, 256 outcat]
    DVE add bias (replicated [128, 256] tile) -> output slab in SBUF

DMA moves 2 MB in / 4 MB out per 32-block chunk with fully contiguous
per-partition runs, which keeps the kernel at the HBM roofline (~24 MB per
core total traffic).
"""

import os
from contextlib import ExitStack

import numpy as np

_B, _N, _L = 1048576, 16, 6
_NCORES = 8
_SPR = 8  # samples packed per 128-wide row (8 * 16 feats = 128)
_P = 128
_BC = _B // _NCORES  # samples per core
_ROWS = _BC // _SPR  # 16384 rows of 128 per core
_A_TOTAL = _ROWS // _P  # 128 transpose blocks per core
_A_CH = 32  # blocks per DMA chunk
_N_CH = _A_TOTAL // _A_CH

# "float32r" = single-pass reduced-precision fp32 matmul (1 cyc/row at
# N>=256 vs 4 cyc/row for exact fp32).  Flip to "float32" if precision
# ever becomes the constraint.
MM_DTYPE = "float32r"
TRACE = False

_SQRT_2HBAR = 2.0

last_run_info = None
_cached = {}


def _run_affine(disp, sq, bs):
    """Evolve the affine map (A, b) with mx = x @ Amx + bmx, in float64.

    Mirrors reference._run_circuit exactly, but on the coefficients of the
    affine map instead of on a batch of samples.
    """
    disp = np.asarray(disp, np.float64)
    sq = np.asarray(sq, np.float64)
    bs = np.asarray(bs, np.float64)
    N = disp.shape[1]
    Amx = _SQRT_2HBAR * np.eye(N)
    Amp = np.zeros((N, N))
    bmx = np.zeros(N)
    bmp = np.zeros(N)
    for l in range(disp.shape[0]):
        a, dphi = disp[l, :, 0], disp[l, :, 1]
        bmx = bmx + _SQRT_2HBAR * a * np.cos(dphi)
        bmp = bmp + _SQRT_2HBAR * a * np.sin(dphi)
        r, sphi = np.abs(sq[l, :, 0]), sq[l, :, 1]
        ch, sh = np.cosh(r), np.sinh(r)
        cp, sp = np.cos(sphi), np.sin(sphi)
        c1, c2, c3 = ch - cp * sh, -sp * sh, ch + cp * sh
        Amx, Amp = Amx * c1[None, :] + Amp * c2[None, :], Amx * c2[None, :] + Amp * c3[None, :]
        bmx, bmp = bmx * c1 + bmp * c2, bmx * c2 + bmp * c3
        for w in range(N - 1):
            th = 1.0 / (1.0 + np.exp(-bs[l, w, 0]))
            bphi = bs[l, w, 1]
            ct, st = np.cos(th), np.sin(th)
            cpb, spb = np.cos(bphi), np.sin(bphi)
            x1, x2 = Amx[:, w].copy(), Amx[:, w + 1].copy()
            p1, p2 = Amp[:, w].copy(), Amp[:, w + 1].copy()
            Amx[:, w] = ct * x1 - cpb * st * x2 - spb * st * p2
            Amx[:, w + 1] = cpb * st * x1 + ct * x2 - spb * st * p1
            Amp[:, w] = spb * st * x2 + ct * p1 - cpb * st * p2
            Amp[:, w + 1] = spb * st * x1 + cpb * st * p1 + ct * p2
            e1, e2 = bmx[w], bmx[w + 1]
            f1, f2 = bmp[w], bmp[w + 1]
            bmx[w] = ct * e1 - cpb * st * e2 - spb * st * f2
            bmx[w + 1] = cpb * st * e1 + ct * e2 - spb * st * f1
            bmp[w] = spb * st * e2 + ct * f1 - cpb * st * f2
            bmp[w + 1] = spb * st * e1 + cpb * st * f1 + ct * f2
    return Amx, bmx, Amp, bmp


def _w_bias(displacements, squeezing, beamsplitter):
    Amx0, bmx0, _, _ = _run_affine(displacements[0], squeezing[0], beamsplitter[0])
    _, _, Amp1, bmp1 = _run_affine(displacements[1], squeezing[1], beamsplitter[1])
    W = np.concatenate([Amx0, Amp1], axis=1)  # [16, 32]
    b = np.concatenate([bmx0, bmp1])  # [32]
    return W, b


def _build_nc(rows):
    import concourse.bass as bass
    import concourse.mybir as mybir
    import concourse.tile as tile
    from concourse import bacc
    from bass_rust import add_dep_helper

    f32 = mybir.dt.float32
    a_total = rows // _P
    a_ch = min(_A_CH, a_total)
    n_ch = a_total // a_ch
    assert n_ch * a_ch == a_total

    nc = bacc.Bacc("TRN2", target_bir_lowering=False, debug=False)
    x_d = nc.dram_tensor("x", [rows, 128], f32, kind="ExternalInput")
    w_d = nc.dram_tensor("wbig", [128, 256], f32, kind="ExternalInput")
    b_d = nc.dram_tensor("biasb", [128, 256], f32, kind="ExternalInput")
    i_d = nc.dram_tensor("ident", [128, 128], f32, kind="ExternalInput")
    o_d = nc.dram_tensor("out", [rows, 256], f32, kind="ExternalOutput")

    xv = x_d[:, :].rearrange("(p a) f -> p a f", p=_P)
    ov = o_d[:, :].rearrange("(p a) g -> p a g", p=_P)

    mm_dt = getattr(mybir.dt, MM_DTYPE)

    # fp32/fp32r matmuls self-load their weights, and the combined LW+MM
    # instruction has room for only ONE sync wait in codegen.  Structure deps
    # so every PE instruction needs at most one semaphore:
    #  - all PSUM->SBUF copies AND the bias-add run on DVE, so a matmul's two
    #    deps (xT ready, PSUM slot release) merge into a single DVE wait
    #  - a warmup transpose absorbs the identity-DMA wait up front, so real
    #    transposes wait only on their chunk's input DMA
    with tile.TileContext(nc) as tc, ExitStack() as ctx:
        consts = ctx.enter_context(tc.tile_pool(name="consts", bufs=1))
        in_pool = ctx.enter_context(tc.tile_pool(name="in_pool", bufs=3))
        out_pool = ctx.enter_context(tc.tile_pool(name="out_pool", bufs=3))
        tp_sb = ctx.enter_context(tc.tile_pool(name="tp_sb", bufs=4))
        warm_psum = ctx.enter_context(
            tc.tile_pool(name="warm_psum", bufs=1, space="PSUM")
        )
        tp_psum = ctx.enter_context(
            tc.tile_pool(name="tp_psum", bufs=3, space="PSUM")
        )
        mm_psum = ctx.enter_context(
            tc.tile_pool(name="mm_psum", bufs=4, space="PSUM")
        )

        w_stage = consts.tile([128, 256], f32)
        w_t = consts.tile([128, 256], mm_dt)
        b_stage = consts.tile([128, 256], f32)
        b_t = consts.tile([128, 256], f32)
        id_t = consts.tile([128, 128], f32)
        nc.sync.dma_start(w_stage[:, :], w_d[:, :])
        nc.vector.tensor_copy(w_t[:, :], w_stage[:, :])
        nc.sync.dma_start(b_stage[:, :], b_d[:, :])
        nc.vector.tensor_copy(b_t[:, :], b_stage[:, :])
        nc.sync.dma_start(id_t[:, :], i_d[:, :])

        warm_ps = warm_psum.tile([128, 128], f32)
        warm = nc.tensor.transpose(warm_ps[:, :], id_t[:, :], id_t[:, :])

        for c in range(n_ch):
            in_t = in_pool.tile([128, a_ch, 128], f32)
            nc.sync.dma_start(in_t[:, :, :], xv[:, c * a_ch : (c + 1) * a_ch, :])
            out_t = out_pool.tile([128, a_ch, 256], f32)
            for a in range(a_ch):
                tp_ps = tp_psum.tile([128, 128], f32)
                tr = nc.tensor.transpose(tp_ps[:, :], in_t[:, a, :], id_t[:, :])
                if a == 0:
                    add_dep_helper(
                        tr.ins, warm.ins, sync=True,
                        reason="PE observes ident DMA via warmup transpose",
                    )
                xT = tp_sb.tile([128, 128], mm_dt)
                nc.vector.tensor_copy(xT[:, :], tp_ps[:, :])
                mm_ps = mm_psum.tile([128, 256], f32)
                nc.tensor.matmul(
                    mm_ps[:, :],
                    xT[:, :],
                    w_t[:, :],
                    start=True,
                    stop=True,
                )
                nc.vector.tensor_add(out_t[:, a, :], mm_ps[:, :], b_t[:, :])
            nc.sync.dma_start(ov[:, c * a_ch : (c + 1) * a_ch, :], out_t[:, :, :])

    nc.compile()
    return nc


def _get_nc(rows):
    key = (rows, MM_DTYPE)
    if key not in _cached:
        _cached[key] = _build_nc(rows)
    return _cached[key]


def kernel(x, displacements, squeezing, beamsplitter):
    global last_run_info
    from concourse.bass_utils import run_bass_kernel_spmd

    x = np.ascontiguousarray(np.asarray(x, dtype=np.float32))
    W, b = _w_bias(displacements, squeezing, beamsplitter)

    w_big = np.zeros((128, 256), np.float32)
    for s in range(_SPR):
        w_big[s * 16 : (s + 1) * 16, s * 32 : (s + 1) * 32] = W
    bias_t = np.broadcast_to(
        np.tile(b.astype(np.float32), _SPR)[None, :], (128, 256)
    ).copy()
    ident = np.eye(128, dtype=np.float32)

    nc = _get_nc(_ROWS)
    in_maps = []
    for c in range(_NCORES):
        shard = x[c * _BC : (c + 1) * _BC].reshape(_ROWS, 128)
        in_maps.append(
            {"x": shard, "wbig": w_big, "biasb": bias_t, "ident": ident}
        )

    res = run_bass_kernel_spmd(
        nc, in_maps, core_ids=list(range(_NCORES)), trace=TRACE
    )
    last_run_info = res
    out = np.concatenate(
        [res.results[c]["out"].reshape(_BC, 2 * _N) for c in range(_NCORES)], axis=0
    )
    return out


# revision 15
# speedup vs baseline: 43826.7678x; 43826.7678x over previous
"""Trainium2 Bass kernel for nn_CVQuantumLayer.

The reference "CV quantum circuit" evolves Gaussian means through
displacement / squeezing / beamsplitter gates.  Every gate is affine in the
means vector (mx, mp) and the initial means are linear in x, so the whole
circuit collapses to an affine map per sample:

    out = concat(mx_circuit0(x), mp_circuit1(x)) = x @ W + b,   W [16, 32]

W and b are computed on host in float64 from the tiny gate parameters; the
heavy [1M, 16] @ [16, 32] + b map runs on 8 NeuronCores, data-parallel over
the batch.

Device dataflow (per core, batch shard of 131072 samples):
  - host passes x TRANSPOSED: xt [16, 131072].  SBUF input tiles are
    [128, n]: partition p = (lane j)*16 + (feature f), where the 8 "lanes"
    are 8 equal slices of the batch.  Every DMA is fully contiguous per
    partition (full bandwidth), and no on-device transpose is needed.
  - weights live as two block-diagonal [128, 128] stationary operands
    (8 lane-copies of W[:, :16] resp. W[:, 16:]), so one matmul with a
    [128, 512] moving x-tile produces 512 samples x 8 lanes x 16 outputs.
  - PSUM -> SBUF + per-partition bias-add goes through scalar-engine
    (half A) and vector-engine (half B) in parallel.
  - output is written transposed (outt [32, 131072], contiguous DMA) and
    un-transposed on host.
"""

from contextlib import ExitStack

import numpy as np

_B, _N, _L = 1048576, 16, 6
_NCORES = 8
_BC = _B // _NCORES  # samples per core = 131072
_LANES = 8
_NSUB = _BC // _LANES  # samples per lane = 16384
_NT = 512  # moving-operand width per matmul (fp32 max, exactly 1 PSUM bank)
_N_CHUNK = 4096  # free-dim per DMA chunk (2 MB per chunk per tensor)

# "float32r" = single-pass reduced-precision fp32 matmul (~1.5e-4 rel err,
# 1 cyc/row); "float32" = exact fp32 (4 cyc/row).
MM_DTYPE = "float32r"
TRACE = False

_SQRT_2HBAR = 2.0

last_run_info = None
_cached = {}


def _run_affine(disp, sq, bs):
    """Evolve the affine map (A, b) with mx = x @ Amx + bmx, in float64.

    Mirrors reference._run_circuit exactly, but on the coefficients of the
    affine map instead of on a batch of samples.
    """
    disp = np.asarray(disp, np.float64)
    sq = np.asarray(sq, np.float64)
    bs = np.asarray(bs, np.float64)
    N = disp.shape[1]
    Amx = _SQRT_2HBAR * np.eye(N)
    Amp = np.zeros((N, N))
    bmx = np.zeros(N)
    bmp = np.zeros(N)
    for l in range(disp.shape[0]):
        a, dphi = disp[l, :, 0], disp[l, :, 1]
        bmx = bmx + _SQRT_2HBAR * a * np.cos(dphi)
        bmp = bmp + _SQRT_2HBAR * a * np.sin(dphi)
        r, sphi = np.abs(sq[l, :, 0]), sq[l, :, 1]
        ch, sh = np.cosh(r), np.sinh(r)
        cp, sp = np.cos(sphi), np.sin(sphi)
        c1, c2, c3 = ch - cp * sh, -sp * sh, ch + cp * sh
        Amx, Amp = Amx * c1[None, :] + Amp * c2[None, :], Amx * c2[None, :] + Amp * c3[None, :]
        bmx, bmp = bmx * c1 + bmp * c2, bmx * c2 + bmp * c3
        for w in range(N - 1):
            th = 1.0 / (1.0 + np.exp(-bs[l, w, 0]))
            bphi = bs[l, w, 1]
            ct, st = np.cos(th), np.sin(th)
            cpb, spb = np.cos(bphi), np.sin(bphi)
            x1, x2 = Amx[:, w].copy(), Amx[:, w + 1].copy()
            p1, p2 = Amp[:, w].copy(), Amp[:, w + 1].copy()
            Amx[:, w] = ct * x1 - cpb * st * x2 - spb * st * p2
            Amx[:, w + 1] = cpb * st * x1 + ct * x2 - spb * st * p1
            Amp[:, w] = spb * st * x2 + ct * p1 - cpb * st * p2
            Amp[:, w + 1] = spb * st * x1 + cpb * st * p1 + ct * p2
            e1, e2 = bmx[w], bmx[w + 1]
            f1, f2 = bmp[w], bmp[w + 1]
            bmx[w] = ct * e1 - cpb * st * e2 - spb * st * f2
            bmx[w + 1] = cpb * st * e1 + ct * e2 - spb * st * f1
            bmp[w] = spb * st * e2 + ct * f1 - cpb * st * f2
            bmp[w + 1] = spb * st * e1 + cpb * st * f1 + ct * f2
    return Amx, bmx, Amp, bmp


def _w_bias(displacements, squeezing, beamsplitter):
    Amx0, bmx0, _, _ = _run_affine(displacements[0], squeezing[0], beamsplitter[0])
    _, _, Amp1, bmp1 = _run_affine(displacements[1], squeezing[1], beamsplitter[1])
    W = np.concatenate([Amx0, Amp1], axis=1)  # [16, 32]
    b = np.concatenate([bmx0, bmp1])  # [32]
    return W, b


def _build_nc(bc):
    import concourse.mybir as mybir
    import concourse.tile as tile
    from concourse import bacc

    f32 = mybir.dt.float32
    mm_dt = getattr(mybir.dt, MM_DTYPE)
    nsub = bc // _LANES
    n_chunk = min(_N_CHUNK, nsub)
    n_ch = nsub // n_chunk
    nt_per_chunk = n_chunk // _NT
    assert n_ch * n_chunk == nsub and nt_per_chunk * _NT == n_chunk

    nc = bacc.Bacc("TRN2", target_bir_lowering=False, debug=False)
    # xt host layout: [128, nsub] with row p = (lane j)*16 + (feature f),
    # column n = position within the lane's batch slice.  Outputs oa/ob:
    # [128, nsub] with row p = j*16 + (output o within the half).
    xt_d = nc.dram_tensor("xt", [128, nsub], mm_dt, kind="ExternalInput")
    wa_d = nc.dram_tensor("wa", [128, 128], mm_dt, kind="ExternalInput")
    wb_d = nc.dram_tensor("wb", [128, 128], mm_dt, kind="ExternalInput")
    ba_d = nc.dram_tensor("ba", [128, 1], f32, kind="ExternalInput")
    bb_d = nc.dram_tensor("bb", [128, 1], f32, kind="ExternalInput")
    oa_d = nc.dram_tensor("oa", [128, nsub], f32, kind="ExternalOutput")
    ob_d = nc.dram_tensor("ob", [128, nsub], f32, kind="ExternalOutput")

    with tile.TileContext(nc) as tc, ExitStack() as ctx:
        consts = ctx.enter_context(tc.tile_pool(name="consts", bufs=1))
        in_pool = ctx.enter_context(tc.tile_pool(name="in_pool", bufs=3))
        outa_pool = ctx.enter_context(tc.tile_pool(name="outa_pool", bufs=3))
        outb_pool = ctx.enter_context(tc.tile_pool(name="outb_pool", bufs=3))
        psa_pool = ctx.enter_context(
            tc.tile_pool(name="psa_pool", bufs=4, space="PSUM")
        )
        psb_pool = ctx.enter_context(
            tc.tile_pool(name="psb_pool", bufs=4, space="PSUM")
        )

        wa_t = consts.tile([128, 128], mm_dt)
        wb_t = consts.tile([128, 128], mm_dt)
        ba_t = consts.tile([128, 1], f32)
        bb_t = consts.tile([128, 1], f32)
        nc.sync.dma_start(wa_t[:, :], wa_d[:, :])
        nc.sync.dma_start(wb_t[:, :], wb_d[:, :])
        nc.sync.dma_start(ba_t[:, :], ba_d[:, :])
        nc.sync.dma_start(bb_t[:, :], bb_d[:, :])

        for c in range(n_ch):
            csl = slice(c * n_chunk, (c + 1) * n_chunk)
            in_t = in_pool.tile([128, n_chunk], mm_dt)
            nc.sync.dma_start(in_t[:, :], xt_d[:, csl])
            outa_t = outa_pool.tile([128, n_chunk], f32)
            outb_t = outb_pool.tile([128, n_chunk], f32)
            for t in range(nt_per_chunk):
                sl = slice(t * _NT, (t + 1) * _NT)
                psa = psa_pool.tile([128, _NT], f32)
                nc.tensor.matmul(
                    psa[:, :], wa_t[:, :], in_t[:, sl], start=True, stop=True
                )
                nc.scalar.add(outa_t[:, sl], psa[:, :], ba_t[:, 0:1])
                psb = psb_pool.tile([128, _NT], f32)
                nc.tensor.matmul(
                    psb[:, :], wb_t[:, :], in_t[:, sl], start=True, stop=True
                )
                nc.vector.tensor_scalar_add(outb_t[:, sl], psb[:, :], bb_t[:, 0:1])
            nc.sync.dma_start(oa_d[:, csl], outa_t[:, :])
            nc.sync.dma_start(ob_d[:, csl], outb_t[:, :])

    nc.compile()
    return nc


def _get_nc(bc):
    key = (bc, MM_DTYPE)
    if key not in _cached:
        _cached[key] = _build_nc(bc)
    return _cached[key]


def _lane_blockdiag(Wh):
    """[16, 16] -> block-diagonal [128, 128] with 8 lane copies."""
    out = np.zeros((128, 128), np.float32)
    for j in range(_LANES):
        out[j * 16 : (j + 1) * 16, j * 16 : (j + 1) * 16] = Wh
    return out


def kernel(x, displacements, squeezing, beamsplitter):
    global last_run_info
    from concourse.bass_utils import run_bass_kernel_spmd

    x = np.asarray(x, dtype=np.float32)
    W, b = _w_bias(displacements, squeezing, beamsplitter)
    W32 = W.astype(np.float32)
    b32 = b.astype(np.float32)

    wa = _lane_blockdiag(W32[:, :16])
    wb = _lane_blockdiag(W32[:, 16:])
    ba = np.tile(b32[:16], _LANES).reshape(128, 1).astype(np.float32)
    bb = np.tile(b32[16:], _LANES).reshape(128, 1).astype(np.float32)

    # [B, 16] -> per-core [128, nsub]: row j*16+f, col n = x[core, j*nsub+n, f]
    xp = np.ascontiguousarray(
        x.reshape(_NCORES, _LANES, _NSUB, 16).transpose(0, 1, 3, 2)
    ).reshape(_NCORES, 128, _NSUB)

    nc = _get_nc(_BC)
    in_maps = [
        {"xt": xp[c], "wa": wa, "wb": wb, "ba": ba, "bb": bb}
        for c in range(_NCORES)
    ]

    res = run_bass_kernel_spmd(
        nc, in_maps, core_ids=list(range(_NCORES)), trace=TRACE
    )
    last_run_info = res
    out = np.empty((_B, 2 * _N), np.float32)
    for c in range(_NCORES):
        # oa/ob rows j*16+o, cols n  ->  out[c*BC + j*nsub + n, o(+16)]
        oa = res.results[c]["oa"].reshape(_LANES, 16, _NSUB)
        ob = res.results[c]["ob"].reshape(_LANES, 16, _NSUB)
        dst = out[c * _BC : (c + 1) * _BC].reshape(_LANES, _NSUB, 2 * _N)
        dst[:, :, :16] = oa.transpose(0, 2, 1)
        dst[:, :, 16:] = ob.transpose(0, 2, 1)
    return out


# revision 29
# speedup vs baseline: 47252.0207x; 1.0782x over previous
"""Trainium2 Bass kernel for nn_CVQuantumLayer.

The reference "CV quantum circuit" evolves Gaussian means through
displacement / squeezing / beamsplitter gates.  Every gate is affine in the
means vector (mx, mp) and the initial means are linear in x, so the whole
circuit collapses to an affine map per sample:

    out = concat(mx_circuit0(x), mp_circuit1(x)) = x @ W + b,   W [16, 32]

W and b are computed on host in float64 from the tiny gate parameters; the
heavy [1M, 16] @ [16, 32] + b map runs on 8 NeuronCores, data-parallel over
the batch.

Device dataflow (per core, batch shard of 131072 samples):
  - host passes x TRANSPOSED: xt [16, 131072].  SBUF input tiles are
    [128, n]: partition p = (lane j)*16 + (feature f), where the 8 "lanes"
    are 8 equal slices of the batch.  Every DMA is fully contiguous per
    partition (full bandwidth), and no on-device transpose is needed.
  - weights live as two block-diagonal [128, 128] stationary operands
    (8 lane-copies of W[:, :16] resp. W[:, 16:]), so one matmul with a
    [128, 512] moving x-tile produces 512 samples x 8 lanes x 16 outputs.
  - PSUM -> SBUF + per-partition bias-add goes through scalar-engine
    (half A) and vector-engine (half B) in parallel.
  - output is written transposed (outt [32, 131072], contiguous DMA) and
    un-transposed on host.
"""

from contextlib import ExitStack

import numpy as np

_B, _N, _L = 1048576, 16, 6
_NCORES = 8
_BC = _B // _NCORES  # samples per core = 131072
_LANES = 8
_NSUB = _BC // _LANES  # samples per lane = 16384
_NT = 512  # moving-operand width per matmul (fp32 max, exactly 1 PSUM bank)
_N_CHUNK = 2048  # free-dim per DMA chunk (1 MB per chunk per tensor)

# "float32" = exact fp32 matmul (bit-identical error envelope to any fp32
# computation, rel err ~2e-7); "float32r" = single-pass reduced-precision
# matmul (~1.2e-4 rel err, ~5% faster end-to-end).
MM_DTYPE = "float32"
TRACE = False

_SQRT_2HBAR = 2.0

last_run_info = None
_cached = {}


def _run_affine(disp, sq, bs):
    """Evolve the affine map (A, b) with mx = x @ Amx + bmx, in float64.

    Mirrors reference._run_circuit exactly, but on the coefficients of the
    affine map instead of on a batch of samples.
    """
    disp = np.asarray(disp, np.float64)
    sq = np.asarray(sq, np.float64)
    bs = np.asarray(bs, np.float64)
    N = disp.shape[1]
    Amx = _SQRT_2HBAR * np.eye(N)
    Amp = np.zeros((N, N))
    bmx = np.zeros(N)
    bmp = np.zeros(N)
    for l in range(disp.shape[0]):
        a, dphi = disp[l, :, 0], disp[l, :, 1]
        bmx = bmx + _SQRT_2HBAR * a * np.cos(dphi)
        bmp = bmp + _SQRT_2HBAR * a * np.sin(dphi)
        r, sphi = np.abs(sq[l, :, 0]), sq[l, :, 1]
        ch, sh = np.cosh(r), np.sinh(r)
        cp, sp = np.cos(sphi), np.sin(sphi)
        c1, c2, c3 = ch - cp * sh, -sp * sh, ch + cp * sh
        Amx, Amp = Amx * c1[None, :] + Amp * c2[None, :], Amx * c2[None, :] + Amp * c3[None, :]
        bmx, bmp = bmx * c1 + bmp * c2, bmx * c2 + bmp * c3
        for w in range(N - 1):
            th = 1.0 / (1.0 + np.exp(-bs[l, w, 0]))
            bphi = bs[l, w, 1]
            ct, st = np.cos(th), np.sin(th)
            cpb, spb = np.cos(bphi), np.sin(bphi)
            x1, x2 = Amx[:, w].copy(), Amx[:, w + 1].copy()
            p1, p2 = Amp[:, w].copy(), Amp[:, w + 1].copy()
            Amx[:, w] = ct * x1 - cpb * st * x2 - spb * st * p2
            Amx[:, w + 1] = cpb * st * x1 + ct * x2 - spb * st * p1
            Amp[:, w] = spb * st * x2 + ct * p1 - cpb * st * p2
            Amp[:, w + 1] = spb * st * x1 + cpb * st * p1 + ct * p2
            e1, e2 = bmx[w], bmx[w + 1]
            f1, f2 = bmp[w], bmp[w + 1]
            bmx[w] = ct * e1 - cpb * st * e2 - spb * st * f2
            bmx[w + 1] = cpb * st * e1 + ct * e2 - spb * st * f1
            bmp[w] = spb * st * e2 + ct * f1 - cpb * st * f2
            bmp[w + 1] = spb * st * e1 + cpb * st * f1 + ct * f2
    return Amx, bmx, Amp, bmp


def _w_bias(displacements, squeezing, beamsplitter):
    Amx0, bmx0, _, _ = _run_affine(displacements[0], squeezing[0], beamsplitter[0])
    _, _, Amp1, bmp1 = _run_affine(displacements[1], squeezing[1], beamsplitter[1])
    W = np.concatenate([Amx0, Amp1], axis=1)  # [16, 32]
    b = np.concatenate([bmx0, bmp1])  # [32]
    return W, b


def _build_nc(bc):
    import concourse.mybir as mybir
    import concourse.tile as tile
    from concourse import bacc

    f32 = mybir.dt.float32
    mm_dt = getattr(mybir.dt, MM_DTYPE)
    nsub = bc // _LANES
    n_chunk = min(_N_CHUNK, nsub)
    # small first chunk (shorter pipeline fill) and small last chunk
    # (shorter drain tail); full-size chunks in between
    if nsub > 2 * n_chunk:
        half = n_chunk // 2
        mid = (nsub - 2 * half) // n_chunk
        chunks = [half] + [n_chunk] * mid + [half] * ((nsub - 2 * half) % n_chunk // half) + [half]
        assert sum(chunks) == nsub, (chunks, nsub)
    else:
        chunks = [n_chunk] * (nsub // n_chunk)
    assert all(ch % _NT == 0 for ch in chunks)

    nc = bacc.Bacc("TRN2", target_bir_lowering=False, debug=False)
    # xt host layout: [128, nsub] with row p = (lane j)*16 + (feature f),
    # column n = position within the lane's batch slice.  Outputs oa/ob:
    # [128, nsub] with row p = j*16 + (output o within the half).
    xt_d = nc.dram_tensor("xt", [128, nsub], mm_dt, kind="ExternalInput")
    wa_d = nc.dram_tensor("wa", [128, 128], mm_dt, kind="ExternalInput")
    wb_d = nc.dram_tensor("wb", [128, 128], mm_dt, kind="ExternalInput")
    ba_d = nc.dram_tensor("ba", [128, 1], f32, kind="ExternalInput")
    bb_d = nc.dram_tensor("bb", [128, 1], f32, kind="ExternalInput")
    oa_d = nc.dram_tensor("oa", [128, nsub], f32, kind="ExternalOutput")
    ob_d = nc.dram_tensor("ob", [128, nsub], f32, kind="ExternalOutput")

    with tile.TileContext(nc) as tc, ExitStack() as ctx:
        consts = ctx.enter_context(tc.tile_pool(name="consts", bufs=1))
        in_pool = ctx.enter_context(tc.tile_pool(name="in_pool", bufs=6))
        outa_pool = ctx.enter_context(tc.tile_pool(name="outa_pool", bufs=4))
        outb_pool = ctx.enter_context(tc.tile_pool(name="outb_pool", bufs=4))
        psa_pool = ctx.enter_context(
            tc.tile_pool(name="psa_pool", bufs=4, space="PSUM")
        )
        psb_pool = ctx.enter_context(
            tc.tile_pool(name="psb_pool", bufs=4, space="PSUM")
        )

        wa_t = consts.tile([128, 128], mm_dt)
        wb_t = consts.tile([128, 128], mm_dt)
        ba_t = consts.tile([128, 1], f32)
        bb_t = consts.tile([128, 1], f32)
        nc.sync.dma_start(wa_t[:, :], wa_d[:, :])
        nc.sync.dma_start(wb_t[:, :], wb_d[:, :])
        nc.sync.dma_start(ba_t[:, :], ba_d[:, :])
        nc.sync.dma_start(bb_t[:, :], bb_d[:, :])

        pos = 0
        for c, ch in enumerate(chunks):
            csl = slice(pos, pos + ch)
            pos += ch
            in_t = in_pool.tile([128, n_chunk], mm_dt, tag="in_t")
            nc.sync.dma_start(in_t[:, :ch], xt_d[:, csl])
            outa_t = outa_pool.tile([128, n_chunk], f32, tag="outa_t")
            outb_t = outb_pool.tile([128, n_chunk], f32, tag="outb_t")
            for t in range(ch // _NT):
                sl = slice(t * _NT, (t + 1) * _NT)
                psa = psa_pool.tile([128, _NT], f32)
                nc.tensor.matmul(
                    psa[:, :], wa_t[:, :], in_t[:, sl], start=True, stop=True
                )
                nc.scalar.add(outa_t[:, sl], psa[:, :], ba_t[:, 0:1])
                psb = psb_pool.tile([128, _NT], f32)
                nc.tensor.matmul(
                    psb[:, :], wb_t[:, :], in_t[:, sl], start=True, stop=True
                )
                nc.vector.tensor_scalar_add(outb_t[:, sl], psb[:, :], bb_t[:, 0:1])
            # output DMAs go out on the ACT HWDGE ring so input loads on the
            # SP ring aren't queued behind them; for the last chunk there is
            # no input left to prefetch, so split the final pair across both
            # rings to halve the drain tail
            last = c == len(chunks) - 1
            nc.scalar.dma_start(oa_d[:, csl], outa_t[:, :ch])
            (nc.sync if last else nc.scalar).dma_start(ob_d[:, csl], outb_t[:, :ch])

    nc.compile()
    return nc


def _get_nc(bc):
    key = (bc, MM_DTYPE)
    if key not in _cached:
        _cached[key] = _build_nc(bc)
    return _cached[key]


def _lane_blockdiag(Wh):
    """[16, 16] -> block-diagonal [128, 128] with 8 lane copies."""
    out = np.zeros((128, 128), np.float32)
    for j in range(_LANES):
        out[j * 16 : (j + 1) * 16, j * 16 : (j + 1) * 16] = Wh
    return out


def kernel(x, displacements, squeezing, beamsplitter):
    global last_run_info
    from concourse.bass_utils import run_bass_kernel_spmd

    x = np.asarray(x, dtype=np.float32)
    W, b = _w_bias(displacements, squeezing, beamsplitter)
    W32 = W.astype(np.float32)
    b32 = b.astype(np.float32)

    wa = _lane_blockdiag(W32[:, :16])
    wb = _lane_blockdiag(W32[:, 16:])
    ba = np.tile(b32[:16], _LANES).reshape(128, 1).astype(np.float32)
    bb = np.tile(b32[16:], _LANES).reshape(128, 1).astype(np.float32)

    # [B, 16] -> per-core [128, nsub]: row j*16+f, col n = x[core, j*nsub+n, f]
    xp = np.ascontiguousarray(
        x.reshape(_NCORES, _LANES, _NSUB, 16).transpose(0, 1, 3, 2)
    ).reshape(_NCORES, 128, _NSUB)

    nc = _get_nc(_BC)
    in_maps = [
        {"xt": xp[c], "wa": wa, "wb": wb, "ba": ba, "bb": bb}
        for c in range(_NCORES)
    ]

    res = run_bass_kernel_spmd(
        nc, in_maps, core_ids=list(range(_NCORES)), trace=TRACE
    )
    last_run_info = res
    out = np.empty((_B, 2 * _N), np.float32)
    for c in range(_NCORES):
        # oa/ob rows j*16+o, cols n  ->  out[c*BC + j*nsub + n, o(+16)]
        oa = res.results[c]["oa"].reshape(_LANES, 16, _NSUB)
        ob = res.results[c]["ob"].reshape(_LANES, 16, _NSUB)
        dst = out[c * _BC : (c + 1) * _BC].reshape(_LANES, _NSUB, 2 * _N)
        dst[:, :, :16] = oa.transpose(0, 2, 1)
        dst[:, :, 16:] = ob.transpose(0, 2, 1)
    return out
